# revision 8
# baseline (speedup 1.0000x reference)
"""Trainium2 Bass kernel for EnhancedMultiHeadAttention (B=4, N=1024, C=1024, H=16).

Sharding over 8 NeuronCores: core c = (batch-pair Bp = c//4, head-quad G = c%4).
Each core computes QKV projections, attention and softmax for its 2 batches x
4 heads (6.4 GFLOP, zero redundancy), then a 4-rank AllGather within each
batch-pair group exchanges attention outputs so each core output-projects its
own 512-token slice of the final result.

Layout decisions:
- All matmul operands bf16 (fp32 matmul is 4x slower on the PE); fp32 PSUM.
- x is pre-transposed on the host (x^T: [chan, tok]) so QKV projections,
  attention and the output projection all contract over the partition dim
  with zero on-device transposes.
- k/v token order is REVERSED so the relative-position bias tile becomes
  B^T[kk, qq] = u_h[kk + qq]: a positive-stride overlapping-window DMA from
  a tiny per-head table u_h[m] = bias_table[min(m, 2*MAX_LEN-2), h].
- Softmax skips max-subtraction (logits ~N(0, 0.11); exp cannot overflow).
  Denominators come free as a 65th ones-column in the AV matmul lhsT.
"""

import sys

if "/opt/trn_rl_repo" not in sys.path:
    sys.path.insert(0, "/opt/trn_rl_repo")

from contextlib import ExitStack

import ml_dtypes
import numpy as np

import concourse.bass as bass
import concourse.tile as tile
from concourse import bacc, mybir
from concourse.bass_utils import run_bass_kernel_spmd

F32 = mybir.dt.float32
BF16 = mybir.dt.bfloat16
BF16_NP = ml_dtypes.bfloat16

B, N, C = 4, 1024, 1024
H, D = 16, 64
MAX_LEN = 1000

BPC = 2  # batches per core
HPC = 4  # heads per core
CPC = HPC * D  # 256 channels per core
TOK = BPC * N  # 2048 tokens per core

_NC_CACHE = {}
TRACE = False
LAST_RESULTS = None


def build_nc(scale: float, taps: bool = False):
    nc = bacc.Bacc(
        "TRN2",
        target_bir_lowering=False,
        debug=False,
        num_devices=8,
        enable_partition_id=True,
    )

    # ---- per-core input shards (host-prepared) ----
    xT = nc.declare_dram_parameter("xT", [C, TOK], BF16, isOutput=False)
    xTr = nc.declare_dram_parameter("xTr", [C, TOK], BF16, isOutput=False)
    wq = nc.declare_dram_parameter("wq", [C, CPC], BF16, isOutput=False)
    wk = nc.declare_dram_parameter("wk", [C, CPC], BF16, isOutput=False)
    wv = nc.declare_dram_parameter("wv", [C, CPC], BF16, isOutput=False)
    wp = nc.declare_dram_parameter("wp", [C, C], BF16, isOutput=False)
    u = nc.declare_dram_parameter("u", [HPC, 2048], BF16, isOutput=False)
    bqs = nc.declare_dram_parameter("bqs", [128, 2], F32, isOutput=False)
    bks = nc.declare_dram_parameter("bks", [128, 2], F32, isOutput=False)
    bvb = nc.declare_dram_parameter("bvb", [128, CPC], BF16, isOutput=False)
    bpb = nc.declare_dram_parameter("bpb", [128, C], BF16, isOutput=False)
    out = nc.declare_dram_parameter("out", [512, C], F32, isOutput=True)
    tap = {}
    if taps:
        tap["qT0"] = nc.declare_dram_parameter("dbg_qT0", [128, TOK], BF16, isOutput=True)
        tap["kT0"] = nc.declare_dram_parameter("dbg_kT0", [128, TOK], BF16, isOutput=True)
        tap["v00"] = nc.declare_dram_parameter("dbg_v00", [128, HPC * 65], BF16, isOutput=True)
        tap["bias"] = nc.declare_dram_parameter("dbg_bias", [128, 2048], BF16, isOutput=True)
        tap["ex"] = nc.declare_dram_parameter("dbg_ex", [128, 2048], BF16, isOutput=True)
        tap["un"] = nc.declare_dram_parameter("dbg_un", [65, 512], BF16, isOutput=True)
        tap["rc"] = nc.declare_dram_parameter("dbg_rc", [16, 512], BF16, isOutput=True)
        tap["att0"] = nc.declare_dram_parameter("dbg_att0", [128, TOK], BF16, isOutput=True)
        tap["gath0"] = nc.declare_dram_parameter("dbg_gath0", [128, 512], BF16, isOutput=True)
        tap["pid"] = nc.declare_dram_parameter("dbg_pid", [1, 2], mybir.dt.uint32, isOutput=True)
        tap["un2"] = nc.declare_dram_parameter("dbg_un2", [65, 512], BF16, isOutput=True)
        tap["bc1"] = nc.declare_dram_parameter("dbg_bc1", [64, 512], BF16, isOutput=True)
        tap["dn"] = nc.declare_dram_parameter("dbg_dn", [16, 512], BF16, isOutput=True)
        tap["bc0"] = nc.declare_dram_parameter("dbg_bc0", [64, 512], BF16, isOutput=True)

    # collective buffers (validated pattern: raw internal DRAM tensors)
    ag_in = nc.dram_tensor("ag_in", [CPC, TOK], BF16)
    ag_out = nc.dram_tensor("ag_out", [4 * CPC, TOK], BF16)

    Exp = mybir.ActivationFunctionType.Exp

    with tile.TileContext(nc) as tc, ExitStack() as octx:
        # ---------- long-lived pools ----------
        wpool = octx.enter_context(tc.tile_pool(name="weights", bufs=1))
        qkpool = octx.enter_context(tc.tile_pool(name="qk", bufs=1))
        vpool = octx.enter_context(tc.tile_pool(name="vtiles", bufs=1))
        aopool = octx.enter_context(tc.tile_pool(name="attout", bufs=1))
        unpool = octx.enter_context(tc.tile_pool(name="unorm", bufs=16))
        drpool = octx.enter_context(tc.tile_pool(name="dram", bufs=1, space="DRAM"))

        denom_d = drpool.tile([16, 512], BF16, tag="denom")
        recip_d = drpool.tile([64, 16, 512], BF16, tag="recip")

        wq_sb = [wpool.tile([128, CPC], BF16, tag=f"wq{i}", name=f"wq{i}") for i in range(8)]
        wk_sb = [wpool.tile([128, CPC], BF16, tag=f"wk{i}", name=f"wk{i}") for i in range(8)]
        wv_sb = [wpool.tile([128, CPC], BF16, tag=f"wv{i}", name=f"wv{i}") for i in range(8)]
        wp_sb = [wpool.tile([128, C], BF16, tag=f"wp{i}", name=f"wp{i}") for i in range(8)]
        bqs_sb = wpool.tile([128, 2], F32, tag="bqs")
        bks_sb = wpool.tile([128, 2], F32, tag="bks")
        bvb_sb = wpool.tile([128, CPC], BF16, tag="bvb")
        bpb_sb = wpool.tile([128, C], BF16, tag="bpb")
        for kt in range(8):
            ks = slice(128 * kt, 128 * kt + 128)
            nc.gpsimd.dma_start(wq_sb[kt][:], wq[ks, :])
            nc.gpsimd.dma_start(wk_sb[kt][:], wk[ks, :])
            nc.gpsimd.dma_start(wv_sb[kt][:], wv[ks, :])
            nc.gpsimd.dma_start(wp_sb[kt][:], wp[ks, :])
        nc.gpsimd.dma_start(bqs_sb[:], bqs[:])
        nc.gpsimd.dma_start(bks_sb[:], bks[:])
        nc.gpsimd.dma_start(bvb_sb[:], bvb[:])
        nc.gpsimd.dma_start(bpb_sb[:], bpb[:])

        # q^T/k^T: [256 chan, 2048 tok] as 2 tiles [128, 2048] (head-pair each)
        qT_sb = [qkpool.tile([128, TOK], BF16, tag=f"qT{i}", name=f"qT{i}") for i in range(2)]
        kT_sb = [qkpool.tile([128, TOK], BF16, tag=f"kT{i}", name=f"kT{i}") for i in range(2)]
        # v (token-reversed rows), per batch: 8 tiles [128, 4*65]; cols 65h..65h+63
        # hold head h's channels, col 65h+64 holds ones (softmax denominator trick)
        v_sb = [
            [vpool.tile([128, HPC * 65], BF16, tag=f"v{b}_{t}", name=f"v{b}_{t}") for t in range(8)]
            for b in range(BPC)
        ]
        for b in range(BPC):
            for tt in range(8):
                v3 = v_sb[b][tt].rearrange("p (h c) -> p h c", c=65)
                nc.vector.memset(v3[:, :, 64:65], 1.0)

        att_sb = [aopool.tile([128, TOK], BF16, tag=f"att{i}", name=f"att{i}") for i in range(2)]

        # ---------- phase B: QKV projections ----------
        with ExitStack() as bctx:
            xpool = bctx.enter_context(tc.tile_pool(name="xT", bufs=8))
            pj = bctx.enter_context(tc.tile_pool(name="pjpsum", bufs=2, space="PSUM"))
            pv = bctx.enter_context(tc.tile_pool(name="pvpsum", bufs=2, space="PSUM"))
            for b in range(BPC):
                xT_b = [xpool.tile([128, N], BF16, tag="xTb", name="xTb") for _ in range(8)]
                xTr_b = [xpool.tile([128, N], BF16, tag="xTrb", name="xTrb") for _ in range(8)]
                for kt in range(8):
                    ks = slice(128 * kt, 128 * kt + 128)
                    ts = slice(N * b, N * b + N)
                    nc.gpsimd.dma_start(xT_b[kt][:], xT[ks, ts])
                    nc.gpsimd.dma_start(xTr_b[kt][:], xTr[ks, ts])
                for ct in range(2):
                    cs = slice(128 * ct, 128 * ct + 128)
                    for qb in range(2):
                        qs = slice(512 * qb, 512 * qb + 512)
                        ps_q = pj.tile([128, 512], F32, tag="psq")
                        ps_k = pj.tile([128, 512], F32, tag="psk")
                        for kt in range(8):
                            nc.tensor.matmul(
                                ps_q[:], wq_sb[kt][:, cs], xT_b[kt][:, qs],
                                start=(kt == 0), stop=(kt == 7),
                            )
                        for kt in range(8):
                            nc.tensor.matmul(
                                ps_k[:], wk_sb[kt][:, cs], xTr_b[kt][:, qs],
                                start=(kt == 0), stop=(kt == 7),
                            )
                        dst = slice(N * b + 512 * qb, N * b + 512 * qb + 512)
                        nc.vector.tensor_scalar_add(
                            qT_sb[ct][:, dst], ps_q[:], bqs_sb[:, ct : ct + 1]
                        )
                        nc.vector.tensor_scalar_add(
                            kT_sb[ct][:, dst], ps_k[:], bks_sb[:, ct : ct + 1]
                        )
                for tt in range(8):
                    ps_v = pv.tile([128, CPC], F32, tag="psv")
                    for kt in range(8):
                        nc.tensor.matmul(
                            ps_v[:],
                            xTr_b[kt][:, 128 * tt : 128 * tt + 128],
                            wv_sb[kt][:],
                            start=(kt == 0), stop=(kt == 7),
                        )
                    v3 = v_sb[b][tt].rearrange("p (h c) -> p h c", c=65)
                    ps3 = ps_v.rearrange("p (h c) -> p h c", c=64)
                    bv3 = bvb_sb.rearrange("p (h c) -> p h c", c=64)
                    nc.vector.tensor_add(v3[:, :, 0:64], ps3[:], bv3[:])

        # ---------- phase C: attention ----------
        un_tiles = {}
        with ExitStack() as cctx:
            bias_pool = cctx.enter_context(tc.tile_pool(name="bias", bufs=8))
            lg_pool = cctx.enter_context(tc.tile_pool(name="logit", bufs=2))
            ex_pool = cctx.enter_context(tc.tile_pool(name="expT", bufs=4))
            epsum = cctx.enter_context(tc.tile_pool(name="epsum", bufs=1, space="PSUM"))
            apsum = cctx.enter_context(tc.tile_pool(name="apsum", bufs=2, space="PSUM"))
            for h in range(HPC):
                ct, hp = h // 2, 64 * (h % 2)
                btile = {}
                for g in range(2):
                    for qb in range(2):
                        t = bias_pool.tile([128, 2048], BF16, tag="bias")
                        src = bass.AP(
                            u,
                            2048 * h + 512 * g + 512 * qb,
                            [[1, 128], [128, 4], [1, 512]],
                        )
                        nc.gpsimd.dma_start(
                            t.rearrange("p (g f) -> p g f", g=4), src
                        )
                        btile[(g, qb)] = t
                        if taps and h == 0 and g == 0 and qb == 0:
                            nc.gpsimd.dma_start(tap["bias"][:], t[:])
                for b in range(BPC):
                    for qb in range(2):
                        qs = slice(N * b + 512 * qb, N * b + 512 * qb + 512)
                        exps = []
                        for g in range(2):
                            pe = epsum.tile([128, 2048], F32, tag="eps")
                            for ktl in range(4):
                                kt = 4 * g + ktl
                                ks = slice(N * b + 128 * kt, N * b + 128 * kt + 128)
                                nc.tensor.matmul(
                                    pe[:, 512 * ktl : 512 * ktl + 512],
                                    kT_sb[ct][hp : hp + 64, ks],
                                    qT_sb[ct][hp : hp + 64, qs],
                                    start=True, stop=True,
                                )
                            lg = lg_pool.tile([128, 2048], F32, tag="lg")
                            nc.vector.tensor_add(lg[:], pe[:], btile[(g, qb)][:])
                            ex = ex_pool.tile([128, 2048], BF16, tag="ex")
                            nc.scalar.activation(ex[:], lg[:], Exp, scale=scale)
                            exps.append(ex)
                        pa = apsum.tile([65, 512], F32, tag="aps")
                        for kt in range(8):
                            nc.tensor.matmul(
                                pa[:],
                                v_sb[b][kt][:, 65 * h : 65 * h + 65],
                                exps[kt // 4][:, 512 * (kt % 4) : 512 * (kt % 4) + 512],
                                start=(kt == 0), stop=(kt == 7),
                            )
                        r = (b * HPC + h) * 2 + qb
                        un = unpool.tile([65, 512], BF16, tag="un")
                        nc.vector.tensor_copy(un[:], pa[:])
                        nc.gpsimd.dma_start(denom_d[r : r + 1, :], un[64:65, :])
                        un_tiles[r] = un
                        if taps and r == 0:
                            nc.gpsimd.dma_start(tap["un"][:], un[:])
                        if taps and r == 2:
                            nc.gpsimd.dma_start(tap["un2"][:], un[:])
                        if taps and h == 0 and b == 0 and qb == 0:
                            nc.gpsimd.dma_start(tap["ex"][:], exps[0][:])

        if taps:
            nc.gpsimd.dma_start(tap["qT0"][:], qT_sb[0][:])
            nc.gpsimd.dma_start(tap["kT0"][:], kT_sb[0][:])
            nc.gpsimd.dma_start(tap["v00"][:], v_sb[0][0][:])
        # ---------- phase D: reciprocal + normalize ----------
        with ExitStack() as dctx:
            npool = dctx.enter_context(tc.tile_pool(name="norm", bufs=1))
            bcpool = dctx.enter_context(tc.tile_pool(name="bcast", bufs=4))
            dn = npool.tile([16, 512], BF16, tag="dn")
            nc.gpsimd.dma_start(dn[:], denom_d[:])
            if taps:
                nc.gpsimd.dma_start(tap["dn"][:], dn[:])
            rc32 = npool.tile([16, 512], F32, tag="rc32")
            nc.vector.reciprocal(rc32[:], dn[:])
            rc16 = npool.tile([16, 512], BF16, tag="rc16")
            nc.vector.tensor_copy(rc16[:], rc32[:])
            if taps:
                nc.gpsimd.dma_start(tap["rc"][:], rc16[:])
            nc.gpsimd.dma_start(recip_d[0], rc16[:])
            for s in [1, 2, 4, 8, 16, 32]:
                nrep = min(s, 64 - s)
                nc.gpsimd.dma_start(recip_d[s : s + nrep], recip_d[0:nrep])
            for b in range(BPC):
                for h in range(HPC):
                    ct, hp = h // 2, 64 * (h % 2)
                    for qb in range(2):
                        r = (b * HPC + h) * 2 + qb
                        bc = bcpool.tile([64, 512], BF16, tag="bc")
                        nc.gpsimd.dma_start(bc[:], recip_d[:, r, :])
                        if taps and r == 0:
                            nc.gpsimd.dma_start(tap["bc0"][:], bc[:])
                        if taps and r == 1:
                            nc.gpsimd.dma_start(tap["bc1"][:], bc[:])
                        dst = att_sb[ct][
                            hp : hp + 64, N * b + 512 * qb : N * b + 512 * qb + 512
                        ]
                        nc.vector.tensor_mul(dst, un_tiles[r][0:64, :], bc[:])

        if taps:
            nc.gpsimd.dma_start(tap["att0"][:], att_sb[0][:])
        # ---------- phase E: AllGather + output projection ----------
        with ExitStack() as ectx:
            gpool = ectx.enter_context(tc.tile_pool(name="gath", bufs=8))
            opool = ectx.enter_context(tc.tile_pool(name="outsb", bufs=2))
            opsum = ectx.enter_context(tc.tile_pool(name="opsum", bufs=2, space="PSUM"))
            for ct in range(2):
                nc.gpsimd.dma_start(
                    ag_in[128 * ct : 128 * ct + 128, :], att_sb[ct][:]
                )
            nc.gpsimd.collective_compute(
                "AllGather",
                mybir.AluOpType.bypass,
                replica_groups=[[0, 1, 2, 3], [4, 5, 6, 7]],
                ins=[ag_in[:]],
                outs=[ag_out[:]],
            )
            pid = nc.gpsimd.partition_id()
            tok0 = (pid % 4) * 512
            gath = [gpool.tile([128, 512], BF16, tag=f"g{i}", name=f"g{i}") for i in range(8)]
            for ct8 in range(8):
                nc.gpsimd.dma_start(
                    gath[ct8][:],
                    ag_out[128 * ct8 : 128 * ct8 + 128, bass.ds(tok0, 512)],
                )
            if taps:
                nc.gpsimd.dma_start(tap["gath0"][:], gath[0][:])
                pidt = gpool.tile([1, 2], mybir.dt.uint32, tag="pidt", name="pidt")
                nc.gpsimd.memset(pidt[:], 0)
                nc.gpsimd.dma_start(tap["pid"][:], pidt[:])
            for ttl in range(4):
                tsl = slice(128 * ttl, 128 * ttl + 128)
                for oc in range(2):
                    ocs = slice(512 * oc, 512 * oc + 512)
                    po = opsum.tile([128, 512], F32, tag="po")
                    for ct8 in range(8):
                        nc.tensor.matmul(
                            po[:], gath[ct8][:, tsl], wp_sb[ct8][:, ocs],
                            start=(ct8 == 0), stop=(ct8 == 7),
                        )
                    ot = opool.tile([128, 512], F32, tag="ot")
                    nc.vector.tensor_add(ot[:], po[:], bpb_sb[:, ocs])
                    nc.gpsimd.dma_start(out[tsl, ocs], ot[:])

    nc.finalize()
    return nc


def _prep_core(c, x, Wq, bq, Wk, bk, Wv, bv, Wp, bp, bias_table):
    Bp, G = c // 4, c % 4
    cs = slice(CPC * G, CPC * G + CPC)
    hs = slice(HPC * G, HPC * G + HPC)

    xb = x[2 * Bp : 2 * Bp + 2]  # [2, N, C]
    xT = np.concatenate([xb[0].T, xb[1].T], axis=1)  # [C, 2N]
    xr = xb[:, ::-1, :]  # token-reversed per batch
    xTr = np.concatenate([xr[0].T, xr[1].T], axis=1)

    # u_h[m] = bias_table[min(m, 2*MAX_LEN-2), h] for the core's 4 heads
    m = np.minimum(np.arange(2048), 2 * MAX_LEN - 2)
    u = bias_table[m][:, hs].T.copy()  # [HPC, 2048]

    bq_s = bq[cs].reshape(2, 128).T.copy()  # [128, 2] col ct
    bk_s = bk[cs].reshape(2, 128).T.copy()

    bf = lambda a: np.ascontiguousarray(a).astype(BF16_NP)
    return {
        "xT": bf(xT),
        "xTr": bf(xTr),
        "wq": bf(Wq[:, cs]),
        "wk": bf(Wk[:, cs]),
        "wv": bf(Wv[:, cs]),
        "wp": bf(Wp),
        "u": bf(u),
        "bqs": np.ascontiguousarray(bq_s, dtype=np.float32),
        "bks": np.ascontiguousarray(bk_s, dtype=np.float32),
        "bvb": bf(np.broadcast_to(bv[cs], (128, CPC))),
        "bpb": bf(np.broadcast_to(bp, (128, C))),
    }


def kernel(
    x, Wq, bq, Wk, bk, Wv, bv, Wp, bp, bias_table, temperature
) -> np.ndarray:
    global LAST_RESULTS
    x = np.asarray(x, dtype=np.float32)
    temp = float(np.clip(np.asarray(temperature).reshape(-1)[0], 0.1, 10.0))
    scale = 1.0 / (np.sqrt(np.float32(C)).item() * temp)

    key = round(scale, 12)
    if key not in _NC_CACHE:
        _NC_CACHE[key] = build_nc(scale)
    nc = _NC_CACHE[key]

    args = [np.asarray(a, dtype=np.float32) for a in (Wq, bq, Wk, bk, Wv, bv, Wp, bp, bias_table)]
    in_maps = [_prep_core(c, x, *args) for c in range(8)]

    res = run_bass_kernel_spmd(nc, in_maps, list(range(8)), trace=TRACE)
    LAST_RESULTS = res

    out = np.empty((B, N, C), dtype=np.float32)
    for c in range(8):
        Bp, G = c // 4, c % 4
        b = 2 * Bp + G // 2
        r0 = 512 * (G % 2)
        out[b, r0 : r0 + 512, :] = res.results[c]["out"]
    return out
